# revision 1
# baseline (speedup 1.0000x reference)
"""Trainium2 Bass kernel for nn_CNN_58222576664743 (e3nn-style GNN message passing).

Strategy (8 NeuronCores):
- Edges sorted by destination node, sharded into 8 contiguous dst ranges
  (core k owns nodes [1024k, 1024k+1024) and the edges pointing into them).
- Per core: radial embedding + spherical harmonics computed per edge
  (edge-major [128, C, *] layout), hidden fcnet activations for all three
  interaction blocks via PE matmuls (feature-major [96, E]), per-edge
  tensor-product weights generated on the TensorEngine and contracted with
  gathered source features on the VectorEngine (broadcast-AP multiplies +
  segmented reduces).
- Segment-sum over destinations via one-hot matmuls accumulated in PSUM
  (edges are dst-sorted and chunk-aligned per 128-node tile).
- Source-feature gathers via per-chunk indirect DMA from a DRAM node table;
  the block-1/2 node tables are built with an 8-core AllGather.
"""
import dataclasses
import math
import sys

import numpy as np

sys.path.insert(0, "/opt/trn_rl_repo")

import concourse.bacc as bacc  # noqa: E402
import concourse.bass as bass  # noqa: E402
import concourse.mybir as mybir  # noqa: E402
import concourse.tile as tile  # noqa: E402
from concourse.bass import AP, IndirectOffsetOnAxis  # noqa: E402
from concourse.bass_utils import run_bass_kernel_spmd  # noqa: E402

F32 = mybir.dt.float32
I32 = mybir.dt.int32
OP = mybir.AluOpType
AF = mybir.ActivationFunctionType

N_NODES = 8192
N_EDGES = 65536
NCORES = 8
NLOC = 1024
NB = 10
MID = 32
HS = 32
HV = 8
INS = 64
OS = 16
HD = 56
MAX_R = 10.0
AVG = N_EDGES / N_NODES
STEP = MAX_R / (NB + 1)
EMB_C = 1.14136 * math.exp(2.0)

S32 = 1.0 / math.sqrt(MID)
SAVG = 1.0 / math.sqrt(AVG)


def _ap(base, off, dims):
    """AP with base's partition dim, custom free dims, extra element offset."""
    return dataclasses.replace(base, offset=base.offset + off, ap=[base.ap[0]] + dims)


def build(C, tile_nchunks, phase=99, repeat=1):
    """Build the SPMD Bass program.

    C: total edge chunks (128 edges each) per core.
    tile_nchunks: list of 8 ints, chunks assigned to each 128-node tile
                  (sum == C).
    """
    EP = C * 128
    G = C // 4
    nc = bacc.Bacc(None, target_bir_lowering=False)

    din = lambda n, s, dt=F32: nc.dram_tensor(n, list(s), dt, kind="ExternalInput")
    xe0_d = din("xe0", [128, C, 64])
    poss_d = din("possrc", [128, C, 3])
    posd_d = din("posdst", [128, C, 3])
    gsrc_d = din("gsrc", [128, C], I32)
    ldst_d = din("ldst", [128, C])
    w1g_d = din("w1g", [128, 96])
    a_d = din("a_all", [128, 2560])
    v1_d = din("vals1", [128, NB])
    v2_d = din("vals2", [128, NB])
    lng_d = din("lng", [128, 192])
    lnb_d = din("lnb", [128, 192])
    iota_d = din("iota", [128, 128])
    id_d = din("ident", [128, 128])
    out_d = nc.dram_tensor("out", [NLOC, 16], F32, kind="ExternalOutput")

    x1_full = nc.dram_tensor("x1_full", [N_NODES, 64], F32, addr_space="Shared")
    x2_full = nc.dram_tensor("x2_full", [N_NODES, 64], F32, addr_space="Shared")

    RG = [list(range(NCORES))]

    with tile.TileContext(nc) as tc:
        with (
            tc.tile_pool(name="main", bufs=1) as mp,
            tc.tile_pool(name="cyc", bufs=2) as cp,
            tc.tile_pool(name="wps", bufs=1, space="PSUM") as wps,
            tc.tile_pool(name="sps", bufs=2, space="PSUM") as sps,
            tc.tile_pool(name="dram", bufs=1, space="DRAM") as dp,
        ):
            # ---------------- constant + input loads ----------------
            czero = mp.tile([128, 1], F32)
            ceps = mp.tile([128, 1], F32)
            nc.vector.memset(czero[:], 0.0)
            nc.vector.memset(ceps[:], 1e-5)
            nc.const_aps.aps[(F32, 0.0)] = czero[:]
            nc.const_aps.aps[(F32, 1e-5)] = ceps[:]
            w1g = mp.tile([128, 96], F32)
            a_all = mp.tile([128, 2560], F32)
            v1 = mp.tile([128, NB], F32)
            v2 = mp.tile([128, NB], F32)
            lng = mp.tile([128, 192], F32)
            lnb = mp.tile([128, 192], F32)
            iota = mp.tile([128, 128], F32)
            ident = mp.tile([128, 128], F32)
            gsrc = mp.tile([128, C], I32)
            ldst = mp.tile([128, C], F32)
            poss = mp.tile([128, C, 3], F32)
            posd = mp.tile([128, C, 3], F32)
            for t, d in [(w1g, w1g_d), (a_all, a_d), (v1, v1_d), (v2, v2_d),
                         (lng, lng_d), (lnb, lnb_d), (iota, iota_d), (id_d2 := ident, id_d),
                         (gsrc, gsrc_d), (ldst, ldst_d), (poss, poss_d), (posd, posd_d)]:
                nc.sync.dma_start(t[:], d[:])

            # gathered source features (block 0 host-gathered, later x1e/x2e)
            for _rep in range(repeat):
              xe = mp.tile([128, C, 64], F32, tag="xe")
              nc.sync.dma_start(xe[:], xe0_d[:])

              GP = phase  # granular phase
              # ---------------- geometry ----------------
              vecd = mp.tile([128, C, 3], F32)
              sqt = mp.tile([128, C, 3], F32)
              len2 = mp.tile([128, C], F32)
              length = mp.tile([128, C], F32)
              rlen = mp.tile([128, C], F32)
              sh = mp.tile([128, C, 3], F32)
              nc.vector.tensor_sub(vecd[:], posd[:], poss[:])
              nc.scalar.square(sqt[:], vecd[:])
              nc.vector.tensor_reduce(len2[:], sqt[:], mybir.AxisListType.X, OP.add)
              nc.scalar.sqrt(length[:], len2[:])
              nc.vector.tensor_scalar_max(rlen[:], length[:], 1e-9)
              nc.vector.reciprocal(rlen[:], rlen[:])
              # sh = sqrt(3) * vec * rlen
              nc.vector.tensor_tensor(
                  out=sh[:], in0=vecd[:],
                  in1=_ap(rlen[:], 0, [[1, C], [0, 3]]), op=OP.mult)
              nc.scalar.mul(sh[:], sh[:], math.sqrt(3.0))

              # radial embedding, edge-major [128, C, NB]
              ul = mp.tile([128, C], F32)
              y1 = mp.tile([128, C, NB], F32, tag="embt1")
              y2 = mp.tile([128, C, NB], F32, tag="embt2")
              e1 = mp.tile([128, C, NB], F32, tag="embt3")
              m1 = mp.tile([128, C, NB], F32, tag="embt4")
              emb = mp.tile([128, C, 32], F32)
              nc.vector.memset(emb[:], 0.0)
              nc.scalar.mul(ul[:], length[:], 1.0 / STEP)
              ulb = _ap(ul[:], 0, [[1, C], [0, NB]])
              nc.vector.tensor_sub(y1[:], ulb, _ap(v1[:], 0, [[0, C], [1, NB]]))
              nc.vector.tensor_sub(y2[:], _ap(v2[:], 0, [[0, C], [1, NB]]), ulb)

              def sus(dst, y, tmpe, tmpm):
                  nc.vector.tensor_scalar_max(tmpe[:], y[:], 1e-20)
                  nc.vector.reciprocal(tmpe[:], tmpe[:])
                  nc.scalar.activation(tmpe[:], tmpe[:], AF.Exp, scale=-1.0)
                  nc.vector.tensor_scalar(tmpm[:], y[:], 0.0, None, OP.is_gt)
                  nc.vector.tensor_tensor(out=dst, in0=tmpe[:], in1=tmpm[:], op=OP.mult)

              sus(y1[:], y1, e1, m1)      # y1 <- sus(u+1)
              sus(y2[:], y2, e1, m1)      # y2 <- sus(1-u)
              nc.vector.tensor_tensor(
                  out=_ap(emb[:], 0, [[32, C], [1, NB]]), in0=y1[:], in1=y2[:],
                  op=OP.mult)

              # ---------------- hidden activations hT_all [96, EP] ----------------
              hT = mp.tile([96, EP], F32)
              if phase >= 2:
                for c in range(C):
                  pt = sps.tile([32, 128], F32, tag="sp")
                  nc.tensor.transpose(pt[:], emb[:, c, :], ident[:])
                  embTc = cp.tile([32, 128], F32, tag="embt")
                  nc.scalar.copy(embTc[:], pt[:])
                  if phase >= 3:
                      ph = sps.tile([96, 128], F32, tag="sp2")
                      nc.tensor.matmul(
                          ph[:], w1g[0:32, :], embTc[:],
                          start=True, stop=True)
                      nc.scalar.activation(hT[:, c * 128:(c + 1) * 128], ph[:], AF.Silu)

              # ---------------- shared helpers ----------------
              msg = mp.tile([128, C, 64], F32)
              svb = mp.tile([128, C, 8], F32)
              nc.vector.memset(msg[:], 0.0)

              NG = 4  # chunks per contraction group

              def contraction(b, acol, U, windows, in1_fn):
                  """Per-edge weight generation + contraction.
                  windows: list of (slot0, nslots, kind) with kind in es/sv/evv.
                  in1_fn(t, s0, ns) -> broadcast AP for the gathered features."""
                  for t in range(C // NG):
                      for (s0, ns, kind) in windows:
                          W = ns * U
                          ps = wps.tile([128, NG, 512], F32, tag="w")
                          for g in range(NG):
                              nc.tensor.matmul(
                                  ps[:, g, 0:W],
                                  hT[32 * b:32 * b + 32,
                                     128 * (NG * t + g):128 * (NG * t + g + 1)],
                                  a_all[32 * b:32 * b + 32,
                                        acol + s0 * U:acol + (s0 + ns) * U],
                                  start=True, stop=True)
                          P = cp.tile([128, NG, 512], F32, tag="pw")
                          if kind == "evv":
                              din_ = [[512, NG], [128, 3], [16, 8], [1, 16]]
                          else:
                              din_ = [[512, NG], [U, ns], [1, U]]
                          nc.vector.tensor_tensor(
                              out=_ap(P[:], 0, din_), in0=_ap(ps[:], 0, din_),
                              in1=in1_fn(t, s0, ns), op=OP.mult)
                          if kind == "es":
                              dst = _ap(msg[:], (NG * t) * 64 + s0, [[64, NG], [1, ns]])
                          elif kind == "sv":
                              dst = _ap(svb[:], (NG * t) * 8 + (s0 - 32), [[8, NG], [1, ns]])
                          else:  # evv -> msg[:, :, 32 + w*3 + i]
                              dst = _ap(msg[:], (NG * t) * 64 + 32, [[64, NG], [1, 3], [3, 8]])
                          nc.vector.tensor_reduce(dst, _ap(P[:], 0, din_),
                                                  mybir.AxisListType.X, OP.add)

              def segsum_ln(blk, width, xl):
                  """one-hot segsum (PSUM accum) + layernorm -> xl [128, 8, 64]."""
                  c0 = 0
                  for j in range(8):
                      oh = cp.tile([128, max(tile_nchunks) * 128], F32, tag="oh")
                      nch = tile_nchunks[j]
                      for i in range(nch):
                          nc.vector.tensor_scalar(
                              _ap(oh[:], i * 128, [[1, 128]]), iota[:],
                              ldst[:, c0 + i:c0 + i + 1], None, OP.is_equal)
                      pt = sps.tile([128, 64], F32, tag="sp")
                      for i in range(nch):
                          nc.tensor.matmul(
                              pt[:], _ap(oh[:], i * 128, [[1, 128]]),
                              msg[:, c0 + i, :],
                              start=(i == 0), stop=(i == nch - 1))
                      c0 += nch
                      nc.scalar.copy(xl[:, j, :], pt[:])
                  # layernorm on xl[:, :, :width]
                  s = mp.tile([128, 8], F32, tag="lns")
                  mu = mp.tile([128, 8], F32, tag="lnm")
                  r = mp.tile([128, 8], F32, tag="lnr")
                  xw = _ap(xl[:], 0, [[64, 8], [1, width]])
                  nc.vector.tensor_reduce(s[:], xw, mybir.AxisListType.X, OP.add)
                  nc.scalar.mul(mu[:], s[:], 1.0 / width)
                  nc.vector.tensor_sub(xw, xw, _ap(mu[:], 0, [[1, 8], [0, width]]))
                  sq = cp.tile([128, 8, 64], F32, tag="lnsq")
                  sqw = _ap(sq[:], 0, [[64, 8], [1, width]])
                  nc.scalar.square(sqw, xw)
                  nc.vector.tensor_reduce(s[:], sqw, mybir.AxisListType.X, OP.add)
                  nc.scalar.activation(r[:], s[:], AF.Sqrt, bias=1e-5, scale=1.0 / width)
                  nc.vector.reciprocal(r[:], r[:])
                  nc.vector.tensor_tensor(out=xw, in0=xw,
                                          in1=_ap(r[:], 0, [[1, 8], [0, width]]),
                                          op=OP.mult)
                  gof = 64 * blk
                  nc.vector.tensor_tensor(
                      out=xw, in0=xw,
                      in1=_ap(lng[:], gof, [[0, 8], [1, width]]), op=OP.mult)
                  nc.vector.tensor_tensor(
                      out=xw, in0=xw,
                      in1=_ap(lnb[:], gof, [[0, 8], [1, width]]), op=OP.add)

              def to_full(xl, bounce, full):
                  dst = dataclasses.replace(
                      bounce[:, :], ap=[[64, 128], [8192, 8], [1, 64]])
                  nc.sync.dma_start(dst, xl[:])
                  nc.gpsimd.collective_compute(
                      "AllGather", OP.bypass, replica_groups=RG,
                      ins=[bounce[:, :]], outs=[full[:, :]])

              def gather(full, dst_tile):
                  for c in range(C):
                      nc.gpsimd.indirect_dma_start(
                          out=dst_tile[:, c, :], out_offset=None, in_=full[:, :],
                          in_offset=IndirectOffsetOnAxis(ap=gsrc[:, c:c + 1], axis=0))

              def dump_and_stop(tl):
                  dsto = dataclasses.replace(
                      out_d[:, :], ap=[[16, 128], [2048, 8], [1, 16]])
                  nc.sync.dma_start(dsto, _ap(tl[:], 0, [[64, 8], [1, 16]]))

              # ---------------- block 0 ----------------
              B0WIN = [(0, 8, "es"), (8, 8, "es"), (16, 8, "es"), (24, 8, "es"),
                       (32, 8, "sv")]
              xeb = lambda t, s0, ns: _ap(xe[:], (NG * t) * 64, [[64, NG], [0, ns], [1, 64]])
              x1l = mp.tile([128, 8, 64], F32, tag="xl")
              nc.vector.memset(x1l[:], 0.0)
              if phase >= 4:
                  contraction(0, 0, 64, B0WIN, xeb)
                  nc.vector.tensor_tensor(
                      out=_ap(msg[:], 32, [[64, C], [3, 8], [1, 3]]),
                      in0=_ap(svb[:], 0, [[8, C], [1, 8], [0, 3]]),
                      in1=_ap(sh[:], 0, [[3, C], [0, 8], [1, 3]]), op=OP.mult)
                  segsum_ln(0, HD, x1l)
              else:
                  nc.vector.tensor_copy(x1l[:, 0, :], xe[:, 0, :])
                  nc.vector.tensor_copy(_ap(x1l[:], 64, [[1, 32]]), _ap(emb[:], 0, [[1, 32]]))
              x1b = dp.tile([NLOC, 64], F32)
              x1e = mp.tile([128, C, 64], F32, tag="xe1")
              if phase >= 5:
                  to_full(x1l, x1b, x1_full)
                  gather(x1_full, x1e)
              else:
                  nc.vector.memset(x1e[:], 0.1)

              # ---------------- block 1 ----------------
              xd = mp.tile([128, C, 40], F32, tag="xd")
              xvc = mp.tile([128, C, 3, 16], F32)
              dtmp = cp.tile([128, C, 8, 3], F32, tag="pw")
              x2l = mp.tile([128, 8, 64], F32, tag="x2l")
              nc.vector.memset(x2l[:], 0.0)
              if phase >= 6:
                  nc.vector.tensor_copy(_ap(xd[:], 0, [[40, C], [1, 32]]),
                                        _ap(x1e[:], 0, [[64, C], [1, 32]]))
                  nc.vector.tensor_tensor(
                      out=dtmp[:],
                      in0=_ap(x1e[:], 32, [[64, C], [3, 8], [1, 3]]),
                      in1=_ap(sh[:], 0, [[3, C], [0, 8], [1, 3]]), op=OP.mult)
                  nc.vector.tensor_reduce(_ap(xd[:], 32, [[40, C], [1, 8]]),
                                          _ap(dtmp[:], 0, [[24, C], [3, 8], [1, 3]]),
                                          mybir.AxisListType.X, OP.add)
                  nc.vector.tensor_copy(_ap(xvc[:], 0, [[48, C], [16, 3], [1, 8]]),
                                        _ap(x1e[:], 32, [[64, C], [1, 3], [3, 8]]))
                  t2 = cp.tile([128, C, 8], F32, tag="lnsq")
                  for i, (jj, kk) in enumerate([(1, 2), (2, 0), (0, 1)]):
                      nc.vector.tensor_tensor(
                          out=_ap(xvc[:], i * 16 + 8, [[48, C], [1, 8]]),
                          in0=_ap(x1e[:], 32 + jj, [[64, C], [3, 8]]),
                          in1=_ap(sh[:], kk, [[3, C], [0, 8]]), op=OP.mult)
                      nc.vector.tensor_tensor(
                          out=t2[:], in0=_ap(x1e[:], 32 + kk, [[64, C], [3, 8]]),
                          in1=_ap(sh[:], jj, [[3, C], [0, 8]]), op=OP.mult)
                      nc.vector.tensor_sub(
                          _ap(xvc[:], i * 16 + 8, [[48, C], [1, 8]]),
                          _ap(xvc[:], i * 16 + 8, [[48, C], [1, 8]]), t2[:])

                  B1WIN = [(0, 8, "es"), (8, 8, "es"), (16, 8, "es"), (24, 8, "es"),
                           (32, 8, "sv")]
                  xdb = lambda t, s0, ns: _ap(xd[:], (NG * t) * 40, [[40, NG], [0, ns], [1, 40]])
                  contraction(1, 0, 40, B1WIN, xdb)
                  xvb = lambda t, s0, ns: _ap(xvc[:], (NG * t) * 48,
                                              [[48, NG], [16, 3], [0, 8], [1, 16]])
                  contraction(1, 1600, 16, [(0, 24, "evv")], xvb)
                  nc.vector.tensor_tensor(
                      out=dtmp[:],
                      in0=_ap(svb[:], 0, [[8, C], [1, 8], [0, 3]]),
                      in1=_ap(sh[:], 0, [[3, C], [0, 8], [1, 3]]), op=OP.mult)
                  nc.vector.tensor_tensor(
                      out=_ap(msg[:], 32, [[64, C], [1, 24]]),
                      in0=_ap(msg[:], 32, [[64, C], [1, 24]]),
                      in1=_ap(dtmp[:], 0, [[24, C], [1, 24]]), op=OP.add)
                  segsum_ln(1, HD, x2l)
              x2b = dp.tile([NLOC, 64], F32)
              x2e = mp.tile([128, C, 64], F32, tag="xe")  # reuse xe slot
              if phase >= 7:
                  to_full(x2l, x2b, x2_full)
                  gather(x2_full, x2e)
              else:
                  nc.vector.memset(x2e[:], 0.1)

              # ---------------- block 2 ----------------
              xol = mp.tile([128, 8, 64], F32, tag="xol")
              nc.vector.memset(xol[:], 0.0)
              if phase >= 8:
                  nc.vector.memset(_ap(msg[:], 16, [[64, C], [1, 48]]), 0.0)
                  xdb2 = lambda t, s0, ns: _ap(xd[:], (NG * t) * 40, [[40, NG], [0, ns], [1, 40]])
                  nc.vector.tensor_copy(_ap(xd[:], 0, [[40, C], [1, 32]]),
                                        _ap(x2e[:], 0, [[64, C], [1, 32]]))
                  nc.vector.tensor_tensor(
                      out=dtmp[:],
                      in0=_ap(x2e[:], 32, [[64, C], [3, 8], [1, 3]]),
                      in1=_ap(sh[:], 0, [[3, C], [0, 8], [1, 3]]), op=OP.mult)
                  nc.vector.tensor_reduce(_ap(xd[:], 32, [[40, C], [1, 8]]),
                                          _ap(dtmp[:], 0, [[24, C], [3, 8], [1, 3]]),
                                          mybir.AxisListType.X, OP.add)
                  contraction(2, 0, 40, [(0, 8, "es"), (8, 8, "es")], xdb2)
                  segsum_ln(2, OS, xol)
                  dump_and_stop(xol)
              elif phase >= 6:
                  dump_and_stop(x2l)
              else:
                  dump_and_stop(x1l)

    nc.compile()
    return nc


def host_prep(inputs):
    pos = np.asarray(inputs["pos"], np.float32)
    z = np.asarray(inputs["z"]).astype(np.int64)
    mol = np.asarray(inputs["mol_id"]).astype(np.int64)
    src = np.asarray(inputs["edge_src"]).astype(np.int64)
    dst = np.asarray(inputs["edge_dst"]).astype(np.int64)
    Ez = np.asarray(inputs["Ez"], np.float32)
    Em = np.asarray(inputs["Em"], np.float32)

    x0 = np.zeros((N_NODES, 64), np.float32)
    x0[:, :48] = Ez[z]
    x0[:, 48:64] = Em[mol]

    # --- edge sharding: sort by dst, chunk-align per 128-node tile ---
    order = np.argsort(dst, kind="stable")
    s_src, s_dst = src[order], dst[order]
    core_edges = []
    for k in range(NCORES):
        sel = (s_dst >= k * NLOC) & (s_dst < (k + 1) * NLOC)
        core_edges.append((s_src[sel], s_dst[sel] - k * NLOC))
    # per node-tile chunk counts (max across cores)
    tile_nchunks = []
    for j in range(8):
        mx = 1
        for k in range(NCORES):
            cnt = int(np.sum((core_edges[k][1] >= j * 128) &
                             (core_edges[k][1] < (j + 1) * 128)))
            mx = max(mx, (cnt + 127) // 128)
        tile_nchunks.append(mx)
    C = sum(tile_nchunks)
    if C % 4:
        tile_nchunks[7] += 4 - (C % 4)
        C = sum(tile_nchunks)
    EP = C * 128

    per_core = []
    for k in range(NCORES):
        es_, ed_ = core_edges[k]
        gsrc = np.zeros(EP, np.int64)
        ldst = np.full(EP, 999.0, np.float32)  # relative-to-tile dst, 999=dummy
        p0 = 0
        for j in range(8):
            m = (ed_ >= j * 128) & (ed_ < (j + 1) * 128)
            n = int(m.sum())
            gsrc[p0:p0 + n] = es_[m]
            ldst[p0:p0 + n] = (ed_[m] - j * 128).astype(np.float32)
            p0 += tile_nchunks[j] * 128
        pos_s = pos[gsrc]
        pos_d = pos_s.copy()
        real = ldst < 998.0
        # recompute dst positions for real edges
        dglob = np.zeros(EP, np.int64)
        p0 = 0
        for j in range(8):
            m = (ed_ >= j * 128) & (ed_ < (j + 1) * 128)
            n = int(m.sum())
            dglob[p0:p0 + n] = ed_[m] + k * NLOC
            p0 += tile_nchunks[j] * 128
        pos_d[real] = pos[dglob[real]]
        pos_s[~real] = 0.0
        pos_d[~real] = 0.0

        def lay(a, w):
            return np.ascontiguousarray(
                a.reshape(C, 128, w).transpose(1, 0, 2)).astype(np.float32)

        per_core.append({
            "xe0": np.ascontiguousarray(
                x0[gsrc].reshape(C, 128, 64).transpose(1, 0, 2)),
            "gsrc": np.ascontiguousarray(
                gsrc.reshape(C, 128).T).astype(np.int32),
            "ldst": np.ascontiguousarray(ldst.reshape(C, 128).T),
            "possrc": lay(pos_s, 3),
            "posdst": lay(pos_d, 3),
        })

    # --- weights, w-major reorder + scale folding ---
    def w1cat():
        w = np.zeros((128, 96), np.float32)
        for b, key in enumerate(["b0_W1", "b1_W1", "b2_W1"]):
            wb = np.asarray(inputs[key], np.float32) * EMB_C
            for g in range(4):
                w[g * 32:g * 32 + NB, b * 32:(b + 1) * 32] = wb
        return w

    a_all = np.zeros((128, 2560), np.float32)
    b0w2 = np.asarray(inputs["b0_W2"], np.float32)
    sc0 = S32 * (1.0 / math.sqrt(INS)) * SAVG
    A0 = np.zeros((32, 40, 64), np.float32)
    A0[:, :32, :] = b0w2[:, :2048].reshape(32, 64, 32).transpose(0, 2, 1) * sc0
    A0[:, 32:, :] = b0w2[:, 2048:].reshape(32, 64, 8).transpose(0, 2, 1) * sc0
    a_all[0:32, :] = A0.reshape(32, 2560)

    b1w2 = np.asarray(inputs["b1_W2"], np.float32)
    w_ss = b1w2[:, 0:1024].reshape(32, HS, HS)
    w_vvs = b1w2[:, 1024:1280].reshape(32, HV, HS)
    w_sv = b1w2[:, 1280:1536].reshape(32, HS, HV)
    w_vs = b1w2[:, 1536:1600].reshape(32, HV, HV)
    w_vvv = b1w2[:, 1600:1664].reshape(32, HV, HV)
    sq2, sq3 = math.sqrt(2.0), math.sqrt(3.0)
    A1a = np.zeros((32, 40, 40), np.float32)
    A1a[:, :32, :32] = w_ss.transpose(0, 2, 1) * (S32 / math.sqrt(HS) / sq2 * SAVG)
    A1a[:, :32, 32:] = w_vvs.transpose(0, 2, 1) * (S32 / math.sqrt(HV) / sq2 / sq3 * SAVG)
    A1a[:, 32:, :32] = w_sv.transpose(0, 2, 1) * (S32 / math.sqrt(HS) / sq3 * SAVG)
    A1b = np.zeros((32, 3, 8, 16), np.float32)
    for i in range(3):
        A1b[:, i, :, :8] = w_vs.transpose(0, 2, 1) * (S32 / math.sqrt(HV) / sq3 * SAVG)
        A1b[:, i, :, 8:] = w_vvv.transpose(0, 2, 1) * (S32 / sq2 / math.sqrt(HV) / sq3 * SAVG)
    a_all[32:64, 0:1600] = A1a.reshape(32, 1600)
    a_all[32:64, 1600:1984] = A1b.reshape(32, 384)

    b2w2 = np.asarray(inputs["b2_W2"], np.float32)
    w_ss2 = b2w2[:, 0:512].reshape(32, HS, OS)
    w_vvs2 = b2w2[:, 512:640].reshape(32, HV, OS)
    A2 = np.zeros((32, 16, 40), np.float32)
    A2[:, :, :32] = w_ss2.transpose(0, 2, 1) * (S32 / math.sqrt(HS) / sq2 * SAVG)
    A2[:, :, 32:] = w_vvs2.transpose(0, 2, 1) * (S32 / math.sqrt(HV) / sq2 / sq3 * SAVG)
    a_all[64:96, 0:640] = A2.reshape(32, 640)

    vals = np.linspace(0.0, MAX_R, NB + 2)[1:-1].astype(np.float32)
    v1 = np.tile(vals / STEP - 1.0, (128, 1)).astype(np.float32)
    v2 = np.tile(vals / STEP + 1.0, (128, 1)).astype(np.float32)

    lng = np.zeros((128, 192), np.float32)
    lnb = np.zeros((128, 192), np.float32)
    for b, (gk, bk, w) in enumerate([("b0_g", "b0_b", HD), ("b1_g", "b1_b", HD),
                                     ("b2_g", "b2_b", OS)]):
        lng[:, 64 * b:64 * b + w] = np.asarray(inputs[gk], np.float32)[None, :]
        lnb[:, 64 * b:64 * b + w] = np.asarray(inputs[bk], np.float32)[None, :]

    shared = {
        "w1g": w1cat(), "a_all": a_all, "vals1": v1, "vals2": v2,
        "lng": lng, "lnb": lnb,
        "iota": np.tile(np.arange(128, dtype=np.float32), (128, 1)),
        "ident": np.eye(128, dtype=np.float32),
    }
    in_maps = [{**shared, **pc} for pc in per_core]
    return in_maps, C, tile_nchunks


_CACHE = {}


def kernel(**inputs):
    import os
    phase = int(os.environ.get("KPHASE", "99"))
    repeat = int(os.environ.get("KREPEAT", "1"))
    in_maps, C, tile_nchunks = host_prep(inputs)
    key = (C, tuple(tile_nchunks), phase, repeat)
    if key not in _CACHE:
        _CACHE[key] = build(C, tile_nchunks, phase, repeat)
    nc = _CACHE[key]
    res = run_bass_kernel_spmd(nc, in_maps, list(range(NCORES)))
    out = np.zeros((N_NODES, 16), np.float32)
    for k in range(NCORES):
        out[k * NLOC:(k + 1) * NLOC, :] = res.results[k]["out"]
    return out

